# revision 12
# baseline (speedup 1.0000x reference)
"""Trainium2 Bass kernel v2 for nn_Decoder: attention+LSTM decoder.

Math (reference):
  k = h_enc @ Wk.T + bk ; v = h_enc @ Wv.T + bv        [B, 8, 32]
  3 decoder steps: q = h @ Wq.T + bq
     score_t = q.k_t/sqrt(32) ; att = softmax_t
     ctx = sum_t att_t v_t ; (h, c) = LSTMCell(ctx, h, c)
  logits_s = h_s @ Wout.T + b_out ; out = log_softmax(logits)   [B, 3, 10]

v2 algebra (host-side folds):
  score_t = rho . x_t  with  rho = 0.5*A.T Hhat + w,
     A = (Wq.T Wk)/sqrt(H), w = (Wk.T bq)/sqrt(H)   (t-indep terms dropped)
  ctx = Wv xbar + bv with xbar = sum_t att_t x_t  -> k and v never computed.
  gates = M1 xbar + 0.5*W_hh Hhat + bg,  M1 = W_ih@Wv, bg = b_ih+b_hh+W_ih@bv
  sigmoid via tanh; factor-2 carries Chat=2c, Hhat=2h.

Layout: feature-major t-packed tiles [128, n]: partition = 32*(t%4)+h,
halves lo (t0-3) / hi (t4-7) at free offsets 0/n in one [128, 2n] tile.
Scores kept COMPACT: per quad of 4 chunks, scq[32*j + t, :] = score_t of
chunk j (rows t<8 of band j); one exp per quad-step instead of 8.
LSTM state packed per quad: cq/hq [128, n] band j = chunk j's Chat/Hhat.
Gates per role (i,f,o,g) packed per quad -> 1 tanh per role per quad-step.
"""

import numpy as np

import concourse.bass as bass
import concourse.bacc as bacc
import concourse.tile as tile
from concourse import mybir
from concourse.bass_utils import run_bass_kernel_spmd

H = 32
HT = 8
FT = 3
OD = 10
N_CORES = 8

BF = mybir.dt.bfloat16
F32 = mybir.dt.float32
AF = mybir.ActivationFunctionType
ALU = mybir.AluOpType

CHUNK = 512          # batch elements per chunk
QUAD = 4             # chunks per packed state quad
GROUP = 16           # chunks per group (phase A/B batching, >=QUAD, %QUAD==0)

# wpack (bf16, [128, WCOLS]) column layout
ID128 = 0            # 128: identity(128) for input transposes
RHO4 = 128           # 128: 0.5*[A,A,A,A] rows replicated per band
CMPT = 256           # 2*32: compact ones-reduce, half h at CMPT+32h
CMPW = 320           # 2*32: compact w-reduce
EXPB = 384           # 2*128: e-broadcast, half h at EXPB+128h
SUMB = 640           # 32: ssum (rows<8 ones, broadcast out)
I32S = 672           # 32: eye(32) per band (t-reduce for ubar)
GX = 704             # 4*32: (M1 gate-c rows).T per band, c in {i,f,o,g}
GH = 832             # 4*32: (0.5*W_hh gate-c rows).T per band
LGW = 960            # 3*32: logits lhsT per step s (cols 10s+o)
BO10 = 1056          # 32: block-ones(10) for softmax sums
WCOLS = 1088

# fpack (f32, [128, FCOLS])
BIASC = 0            # 4 cols: gate ACT bias per role c
BOUTC = 4            # 1 col: b_out pattern (rows 32j+10s+o), -30 on pad rows
WVEC = 5             # 1 col: w replicated per band (pt STT scalar)
IDT128 = 6           # 128: f32 eye(128) (output transposes)
FCOLS = 134

GATE_SL = (slice(0, 32), slice(32, 64), slice(96, 128), slice(64, 96))  # i,f,o,g


def _pack_weights(Wq, bq, Wk, bk, Wv, bv, W_ih, b_ih, W_hh, b_hh, W_out, b_out):
    Wq, bq, Wk, bk, Wv, bv, W_ih, b_ih, W_hh, b_hh, W_out, b_out = [
        np.asarray(a, np.float32) for a in
        (Wq, bq, Wk, bk, Wv, bv, W_ih, b_ih, W_hh, b_hh, W_out, b_out)]
    s = 1.0 / np.sqrt(np.float32(H))
    A = (Wq.T @ Wk) * s                    # [32,32]
    w = (Wk.T @ bq) * s                    # [32]
    M1 = W_ih @ Wv                         # [128,32]
    bg = b_ih + b_hh + W_ih @ bv           # [128] in (i,f,g,o) order

    wp = np.zeros((128, WCOLS), np.float32)
    wp[:, ID128:ID128 + 128] = np.eye(128)
    for r in range(4):
        P = slice(32 * r, 32 * r + 32)
        for c in range(4):
            wp[P, RHO4 + 32 * c:RHO4 + 32 * c + 32] = 0.5 * A
        for hf in range(2):
            wp[P, CMPT + 32 * hf + (4 * hf + r)] = 1.0
            wp[P, CMPW + 32 * hf + (4 * hf + r)] = w
            for c in range(4):
                wp[32 * r + (4 * hf + c),
                   EXPB + 128 * hf + 32 * c:EXPB + 128 * hf + 32 * c + 32] = 1.0
        wp[32 * r:32 * r + 8, SUMB:SUMB + 32] = 1.0
        wp[P, I32S:I32S + 32] = np.eye(32)
        for c, gsl in enumerate(GATE_SL):
            wp[P, GX + 32 * c:GX + 32 * c + 32] = M1[gsl].T
            wp[P, GH + 32 * c:GH + 32 * c + 32] = (0.5 * W_hh[gsl]).T
        for st in range(FT):
            for o in range(OD):
                wp[P, LGW + 32 * st + OD * st + o] = 0.5 * W_out[o]
        bo = np.zeros((32, 32), np.float32)
        for kk in range(30):
            for oo in range(30):
                if kk // OD == oo // OD:
                    bo[kk, oo] = 1.0
        bo[30, 30] = 1.0
        bo[31, 31] = 1.0
        wp[P, BO10:BO10 + 32] = bo

    fp = np.zeros((128, FCOLS), np.float32)
    for r in range(4):
        P = slice(32 * r, 32 * r + 32)
        for c in range(3):
            fp[P, BIASC + c] = 0.5 * bg[GATE_SL[c]]
        fp[P, BIASC + 3] = bg[GATE_SL[3]]
        bout = np.full(32, -30.0, np.float32)
        for st in range(FT):
            bout[OD * st:OD * st + OD] = b_out
        fp[P, BOUTC] = bout
        fp[P, WVEC] = w
    fp[:, IDT128:IDT128 + 128] = np.eye(128)
    return wp, fp


def build_program(Bshard: int) -> bass.Bass:
    assert Bshard % (QUAD * CHUNK) == 0
    nchunks = Bshard // CHUNK
    nc = bacc.Bacc(trn_type="TRN2")
    x_d = nc.declare_dram_parameter("h_enc", [Bshard, HT, H], F32, isOutput=False)
    wp_d = nc.declare_dram_parameter("wpack", [128, WCOLS], BF, isOutput=False)
    fp_d = nc.declare_dram_parameter("fpack", [128, FCOLS], F32, isOutput=False)
    out_d = nc.declare_dram_parameter("out", [Bshard, FT, OD], F32, isOutput=True)
    with tile.TileContext(nc) as tc:
        _body(nc, tc, x_d, wp_d, fp_d, out_d, nchunks, CHUNK)
    _split_matmul_waits(nc)
    nc.compile()
    return nc


def _split_matmul_waits(nc):
    """Walrus instruction structs fit one sync wait; move extras onto
    same-engine no-ops (each carrying a single wait) inserted just before."""
    for b in nc.m.functions[0].blocks:
        new = []
        for ins in b.instructions:
            si = ins.sync_info
            if (si is not None and len(si.on_wait) > 1
                    and not isinstance(ins, (mybir.InstEventSemaphore,
                                             mybir.InstNoOp))):
                for w in si.on_wait[:-1]:
                    nop = mybir.InstNoOp(
                        name=nc.get_next_instruction_name(), ins=[], outs=[],
                        engine=ins.engine,
                        sync_info=mybir.SyncInfo(on_wait=[w], on_update=[]))
                    nc.register_instruction(nop)
                    new.append(nop)
                ins.sync_info = mybir.SyncInfo(
                    on_wait=[si.on_wait[-1]], on_update=list(si.on_update))
            new.append(ins)
        b.instructions[:] = new


def _body(nc, tc, x_d, wp_d, fp_d, out_d, nchunks, n):
    from contextlib import ExitStack
    ctx = ExitStack()
    with ctx:
        singles = ctx.enter_context(tc.tile_pool(name="singles", bufs=1))
        sb_xb = ctx.enter_context(tc.tile_pool(name="sb_xb", bufs=5))
        sb_xt = ctx.enter_context(tc.tile_pool(name="sb_xt", bufs=GROUP + 10))
        sb_e = ctx.enter_context(tc.tile_pool(name="sb_e", bufs=6))
        sb_eo = ctx.enter_context(tc.tile_pool(name="sb_eo", bufs=3))
        sb_at = ctx.enter_context(tc.tile_pool(name="sb_at", bufs=10))
        sb_pt = ctx.enter_context(tc.tile_pool(name="sb_pt", bufs=24))
        sb_m = ctx.enter_context(tc.tile_pool(name="sb_m", bufs=2))
        sb_rho = ctx.enter_context(tc.tile_pool(name="sb_rho", bufs=3))
        sb_rs = ctx.enter_context(tc.tile_pool(name="sb_rs", bufs=4))
        sb_xq = ctx.enter_context(tc.tile_pool(name="sb_xq", bufs=6))
        sb_tg = ctx.enter_context(tc.tile_pool(name="sb_tg", bufs=5))
        sb_cq = ctx.enter_context(tc.tile_pool(name="sb_cq", bufs=GROUP // QUAD + 1))
        sb_hq = ctx.enter_context(tc.tile_pool(name="sb_hq", bufs=3 * (GROUP // QUAD) + 1))
        sb_ph = ctx.enter_context(tc.tile_pool(name="sb_ph", bufs=GROUP // QUAD + 1))
        sb_ph2 = ctx.enter_context(tc.tile_pool(name="sb_ph2", bufs=2))
        sb_oc = ctx.enter_context(tc.tile_pool(name="sb_oc", bufs=2))
        ps_x = ctx.enter_context(tc.tile_pool(name="ps_x", bufs=2, space="PSUM"))
        ps_f = ctx.enter_context(tc.tile_pool(name="ps_f", bufs=6, space="PSUM"))

        wp = singles.tile([128, WCOLS], BF)
        nc.sync.dma_start(out=wp, in_=wp_d[:, :])
        fp = singles.tile([128, FCOLS], F32)
        nc.sync.dma_start(out=fp, in_=fp_d[:, :])

        ident = wp[:, ID128:ID128 + 128]
        nquads = GROUP // QUAD

        def emit_dma(ci):
            xb = sb_xb.tile([128, 4, 256], BF, tag="xb")
            xv = x_d[ci * n:(ci + 1) * n].rearrange(
                "(i p) t h -> p i (t h)", p=128)
            nc.gpsimd.dma_start(out=xb, in_=xv)
            return xb

        def emit_transpose(xb):
            xp = ps_x.tile([128, 2 * n], BF, tag="xp")
            for hf in range(2):
                for i in range(4):
                    nc.tensor.transpose(
                        xp[:, n * hf + 128 * i:n * hf + 128 * i + 128],
                        xb[:, i, 128 * hf:128 * hf + 128],
                        ident)
            xt = sb_xt.tile([128, 2 * n], BF, tag="xt")
            nc.vector.tensor_copy(
                xt[:, :].bitcast(mybir.dt.int32),
                xp[:, :].bitcast(mybir.dt.int32))
            return xt

        def emit_phase_a(ci):
            return emit_transpose(emit_dma(ci))

        PREFETCH = 8        # next-group chunks transposed during current steps

        g0 = 0
        xts = None
        pending = []        # prefetched xt tiles for the next group
        while g0 < nchunks:
            gsz = min(GROUP, nchunks - g0)
            gq = gsz // QUAD
            # ---------------- phase A (whatever wasn't prefetched) -------
            xts = list(pending)
            pending = []
            for cj in range(len(xts), gsz):
                xts.append(emit_phase_a(g0 + cj))
            next_g0 = g0 + gsz
            next_sz = min(GROUP, nchunks - next_g0) if next_g0 < nchunks else 0
            nprefetch = min(PREFETCH, next_sz)
            pf_xbs = {}
            pf_iter = iter(range(nprefetch))

            # ---------------- recurrent steps (stage-major) ----------------
            hq_all = {}
            cq_prev = {}
            for s in range(1, FT + 1):
                # prefetch DMAs for the transposes this step will emit pre-S8
                npf = (nprefetch + FT - 1) // FT
                for cj in range(npf * (s - 1), min(npf * s, nprefetch)):
                    pf_xbs[cj] = emit_dma(next_g0 + cj)

                # S1: rho MMs + rho copy (ACT, +w bias) + pt TTs
                pt = {}
                if s > 1:
                    for q in range(gq):
                        jj = [q * QUAD + j for j in range(QUAD)]
                        hq_prev = hq_all[(q, s - 1)]
                        rps = []
                        for j in range(QUAD):
                            sj = 32 * ((j + 2) % QUAD)
                            rp = ps_f.tile([128, n], F32, tag="f")
                            nc.tensor.matmul(
                                rp[:, :], wp[sj:sj + 32, RHO4:RHO4 + 128],
                                hq_prev[sj:sj + 32, :],
                                start=True, stop=True,
                                tile_position=(sj, 0), skip_group_check=True)
                            rps.append(rp)
                        for j in range(QUAD):
                            rsb = sb_rho.tile([128, n], BF, tag="rsb")
                            nc.scalar.activation(
                                out=rsb, in_=rps[j], func=AF.Identity,
                                bias=fp[:, WVEC:WVEC + 1])
                            for hf in range(2):
                                t = sb_pt.tile([128, n], BF, tag="pt")
                                nc.vector.tensor_tensor(
                                    out=t, in0=rsb,
                                    in1=xts[jj[j]][:, n * hf:n * hf + n],
                                    op=ALU.mult)
                                pt[(q, j, hf)] = t

                # S2: score volleys
                scqs = []
                for q in range(gq):
                    jj = [q * QUAD + j for j in range(QUAD)]
                    scq = ps_f.tile([128, n], F32, tag="f")
                    for k in range(2):
                        for j in range(QUAD):
                            sj = 32 * ((j + 2) % QUAD)
                            if s > 1:
                                lh = wp[0:128, CMPT + 32 * k:CMPT + 32 * k + 32]
                                rh = pt[(q, j, k)][:, :]
                            else:
                                lh = wp[0:128, CMPW + 32 * k:CMPW + 32 * k + 32]
                                rh = xts[jj[j]][:, n * k:n * k + n]
                            nc.tensor.matmul(
                                scq[sj:sj + 32, :], lh, rh,
                                start=(k == 0), stop=(k == 1),
                                tile_position=(0, sj), skip_group_check=True)
                    scqs.append(scq)

                # S3: exp
                escs = []
                for q in range(gq):
                    esc = sb_e.tile([128, n], BF, tag="esc")
                    nc.scalar.activation(out=esc, in_=scqs[q], func=AF.Exp)
                    escs.append(esc)

                # S4: ssum + recip
                rss = []
                for q in range(gq):
                    ssq = ps_f.tile([128, n], F32, tag="f")
                    for j in range(QUAD):
                        sj = 32 * ((j + 2) % QUAD)
                        nc.tensor.matmul(
                            ssq[32 * j:32 * j + 32, :],
                            wp[sj:sj + 32, SUMB:SUMB + 32],
                            escs[q][sj:sj + 32, :],
                            start=True, stop=True,
                            tile_position=(sj, 32 * j), skip_group_check=True)
                    rs = sb_rs.tile([128, n], F32, tag="rs")
                    nc.vector.reciprocal_approx_fast(out=rs, in_=ssq)
                    rss.append(rs)

                # S5: e-broadcast + at products + ubar reduce + xbar (per quad)
                xqs = []
                for q in range(gq):
                    jj = [q * QUAD + j for j in range(QUAD)]
                    at = {}
                    for hf in range(2):
                        for j in range(QUAD):
                            sj = 32 * ((j + 2) % QUAD)
                            ebp = ps_f.tile([128, n], F32, tag="f")
                            nc.tensor.matmul(
                                ebp[:, :],
                                wp[sj:sj + 32,
                                   EXPB + 128 * hf:EXPB + 128 * hf + 128],
                                escs[q][sj:sj + 32, :],
                                start=True, stop=True,
                                tile_position=(sj, 0), skip_group_check=True)
                            t = sb_at.tile([128, n], BF, tag="at")
                            nc.vector.tensor_tensor(
                                out=t, in0=ebp,
                                in1=xts[jj[j]][:, n * hf:n * hf + n],
                                op=ALU.mult)
                            at[(j, hf)] = t
                    ubq = ps_f.tile([128, n], F32, tag="f")
                    for hf in range(2):
                        for j in range(QUAD):
                            nc.tensor.matmul(
                                ubq[32 * j:32 * j + 32, :],
                                wp[0:128, I32S:I32S + 32], at[(j, hf)][:, :],
                                start=(hf == 0), stop=(hf == 1),
                                tile_position=(0, 32 * j), skip_group_check=True)
                    xq = sb_xq.tile([128, n], BF, tag="xq")
                    nc.vector.tensor_tensor(out=xq, in0=ubq, in1=rss[q],
                                            op=ALU.mult)
                    xqs.append(xq)

                # S7: gates + tanh
                tgs = {}
                for q in range(gq):
                    hq_prev = hq_all.get((q, s - 1))
                    gps = []
                    for _gi in range(4):
                        gp_t = ps_f.tile([128, n], F32, tag="f")
                        gps.append(gp_t)
                    for c in range(4):
                        for j in range(QUAD):
                            sj = 32 * ((j + 2) % QUAD)
                            nc.tensor.matmul(
                                gps[c][sj:sj + 32, :],
                                wp[32 * j:32 * j + 32, GX + 32 * c:GX + 32 * c + 32],
                                xqs[q][32 * j:32 * j + 32, :],
                                start=True, stop=(s == 1),
                                tile_position=(32 * j, sj),
                                skip_group_check=True)
                    if s > 1:
                        for c in range(4):
                            for j in range(QUAD):
                                sj = 32 * ((j + 2) % QUAD)
                                nc.tensor.matmul(
                                    gps[c][sj:sj + 32, :],
                                    wp[sj:sj + 32,
                                       GH + 32 * c:GH + 32 * c + 32],
                                    hq_prev[sj:sj + 32, :],
                                    start=False, stop=True,
                                    tile_position=(sj, sj),
                                    skip_group_check=True)
                    for c in range(4):
                        t = sb_tg.tile([128, n], BF, tag=f"tg{c}")
                        if c < 3:
                            nc.scalar.activation(
                                out=t, in_=gps[c], func=AF.Tanh, scale=0.5,
                                bias=fp[:, BIASC + c:BIASC + c + 1])
                        else:
                            nc.scalar.activation(
                                out=t, in_=gps[c], func=AF.Tanh,
                                bias=fp[:, BIASC + 3:BIASC + 4])
                        tgs[(q, c)] = t

                # next group's transposes fill the PE-idle window of S8
                for _ in range(npf):
                    cj = next(pf_iter, None)
                    if cj is not None:
                        pending.append(emit_transpose(pf_xbs.pop(cj)))

                # S8: LSTM elementwise
                for q in range(gq):
                    tg = [tgs[(q, c)] for c in range(4)]
                    cq_new = sb_cq.tile([128, n], BF, tag="cq")
                    if s == 1:
                        nc.vector.scalar_tensor_tensor(
                            out=cq_new, in0=tg[0], scalar=1.0, in1=tg[3],
                            op0=ALU.add, op1=ALU.mult)
                    else:
                        m1 = sb_m.tile([128, n], BF, tag="m1")
                        nc.vector.scalar_tensor_tensor(
                            out=m1, in0=tg[0], scalar=1.0, in1=tg[3],
                            op0=ALU.add, op1=ALU.mult)
                        u2 = sb_m.tile([128, n], BF, tag="u2")
                        nc.gpsimd.tensor_tensor(
                            out=u2, in0=tg[1], in1=cq_prev[q], op=ALU.mult)
                        m2 = sb_m.tile([128, n], BF, tag="m2")
                        nc.gpsimd.tensor_tensor(
                            out=m2, in0=u2, in1=cq_prev[q], op=ALU.add)
                        nc.vector.scalar_tensor_tensor(
                            out=cq_new, in0=m2, scalar=0.5, in1=m1,
                            op0=ALU.mult, op1=ALU.add)
                    cq_prev[q] = cq_new

                    tcq = sb_tg.tile([128, n], BF, tag="tcq")
                    nc.scalar.activation(out=tcq, in_=cq_new, func=AF.Tanh,
                                         scale=0.5)
                    u3 = sb_m.tile([128, n], BF, tag="u3")
                    nc.gpsimd.tensor_tensor(out=u3, in0=tg[2], in1=tcq,
                                            op=ALU.mult)
                    hq = sb_hq.tile([128, n], BF, tag="hq")
                    nc.gpsimd.tensor_tensor(out=hq, in0=u3, in1=tcq,
                                            op=ALU.add)
                    hq_all[(q, s)] = hq

            # ---------------- phase B ----------------
            sums = []
            lgss = []
            for q in range(gq):
                lgq = ps_f.tile([128, n], F32, tag="f")
                for s in range(1, FT + 1):
                    for j in range(QUAD):
                        sj = 32 * ((j + 2) % QUAD)
                        nc.tensor.matmul(
                            lgq[32 * j:32 * j + 32, :],
                            wp[sj:sj + 32, LGW + 32 * (s - 1):LGW + 32 * s],
                            hq_all[(q, s)][sj:sj + 32, :],
                            start=(s == 1), stop=(s == FT),
                            tile_position=(sj, 32 * j), skip_group_check=True)
                eoq = sb_eo.tile([128, n], BF, tag="eoq")
                nc.scalar.activation(out=eoq, in_=lgq, func=AF.Exp,
                                     bias=fp[:, BOUTC:BOUTC + 1])
                lgs = sb_ph.tile([128, n], F32, tag="lgs")
                nc.scalar.activation(out=lgs, in_=lgq, func=AF.Identity,
                                     bias=fp[:, BOUTC:BOUTC + 1])
                lgss.append(lgs)
                soq = ps_f.tile([128, n], F32, tag="f")
                for j in range(QUAD):
                    nc.tensor.matmul(
                        soq[32 * j:32 * j + 32, :],
                        wp[32 * j:32 * j + 32, BO10:BO10 + 32],
                        eoq[32 * j:32 * j + 32, :],
                        start=True, stop=True,
                        tile_position=(32 * j, 32 * j), skip_group_check=True)
                sm = sb_ph.tile([128, n], F32, tag="sm")
                nc.vector.tensor_copy(sm, soq)
                sums.append(sm)

            for q in range(gq):
                ls = sb_ph2.tile([128, n], F32, tag="ls")
                nc.scalar.activation(out=ls, in_=sums[q], func=AF.Ln)
                res = sb_ph2.tile([128, n], F32, tag="res")
                nc.gpsimd.tensor_sub(out=res, in0=lgss[q], in1=ls)
                otp = ps_f.tile([128, 4 * 128], F32, tag="f")
                for bb in range(4):
                    nc.tensor.transpose(
                        otp[:, 128 * bb:128 * bb + 128],
                        res[:, 128 * bb:128 * bb + 128],
                        fp[:, IDT128:IDT128 + 128])
                oc = sb_oc.tile([128, 4 * 128], F32, tag="oc")
                nc.vector.tensor_copy(
                    oc.rearrange("p (j b so) -> p b j so", j=4, b=4),
                    otp.rearrange("p (b j so) -> p b j so", b=4, j=4))
                oc4 = oc.rearrange("p (j b so) -> p j b so", j=4, b=4)
                base = (g0 + q * QUAD) * n
                ov = out_d[base:base + QUAD * n, :, :].rearrange(
                    "(j b p) s o -> p j b (s o)", p=128, b=4)
                nc.sync.dma_start(out=ov, in_=oc4[:, :, :, 0:FT * OD])
            g0 += gsz


_PROGRAM_CACHE: dict[int, bass.Bass] = {}
_LAST_EXEC_NS = None
_LAST_RESULTS = None


def _get_program(Bshard: int) -> bass.Bass:
    if Bshard not in _PROGRAM_CACHE:
        _PROGRAM_CACHE[Bshard] = build_program(Bshard)
    return _PROGRAM_CACHE[Bshard]


def kernel(**inputs) -> np.ndarray:
    import ml_dtypes
    h_enc = np.asarray(inputs["h_enc"], np.float32)
    B = h_enc.shape[0]
    Bshard = B // N_CORES
    wp, fpk = _pack_weights(
        inputs["Wq"], inputs["bq"], inputs["Wk"], inputs["bk"],
        inputs["Wv"], inputs["bv"], inputs["W_ih"], inputs["b_ih"],
        inputs["W_hh"], inputs["b_hh"], inputs["W_out"], inputs["b_out"])
    wp_bf = wp.astype(ml_dtypes.bfloat16)
    nc = _get_program(Bshard)
    in_maps = []
    for c in range(N_CORES):
        in_maps.append({
            "h_enc": np.ascontiguousarray(h_enc[c * Bshard:(c + 1) * Bshard]),
            "wpack": wp_bf,
            "fpack": fpk,
        })
    import os
    trace = bool(os.environ.get("BASS_TRACE"))
    res = run_bass_kernel_spmd(nc, in_maps, list(range(N_CORES)), trace=trace)
    global _LAST_EXEC_NS, _LAST_RESULTS
    _LAST_EXEC_NS = res.exec_time_ns
    _LAST_RESULTS = res
    outs = [np.asarray(res.results[c]["out"]).reshape(Bshard, FT, OD)
            for c in range(N_CORES)]
    return np.concatenate(outs, axis=0).astype(np.float32)
